# revision 115
# baseline (speedup 1.0000x reference)
"""Distributed single-head causal attention on 8 TRN2 NeuronCores.

Sharding: pair (2b, 2b+1) handles batch b. Query blocks of 512 rows are
assigned for balance: core A=2b gets global blocks {0, 3} (local slots 0,1),
core B=2b+1 gets blocks {1, 2}. Every core computes Q,K,V for its own 1024
rows from bf16 host-packed x (read exactly once globally; 12 interleaved
DMAs keep delivery just ahead of the matmul loop), then the pair AllGathers
K (f32r, column-halves pipelined behind their Act/DVE bias-adds) and
V-augmented (bf16, ones column baked in; halves on the Pool SWDGE so they
never contend with the exp-critical K chain on the HWDGE).

Attention uses TRANSPOSED logits: for a 512-wide t-block and each 128-row
s-chunk, logitsT[s,t] = matmul(lhsT=kT_chunk, rhs=qT_block) with contract
width 66: rows 0:64 are the qkv channels, rows 64:66 fold the per-(slot,
chunk) causal kill bias into the matmul (kT rows 64,65 hold cbias values,
qT rows 64,65 hold slot indicator ones). exp then needs no bias operand, so
two adjacent prefix s-chunks share one [128,1024] activation — the Act
engine's exp stream is the attention-phase bottleneck and runs gap-free
(8 parked p_sb bufs let AVs lag the gathered V). The exp output is bf16
with s on partitions; AV accumulates outT[65, t] via
  matmul(lhsT=v_aug[128s, 65], rhs=P[128s, 512t])
where v_aug's 65th column of ones yields the softmax denominator for free.

SPMD uniformity over the causal structure:
  - slot0 runs 4 prefix s-chunks, slot1 runs 12; chunks past a core's causal
    limit are killed by cbias = -1e30 folded into the logits matmul.
  - the 4 diagonal s-chunks per slot read the core's OWN local K/V at static
    local addresses (identical on both cores of a pair); the triangular mask
    is one static [128, 1024] buffer sliced 4 ways, added in place on DVE
    over only the masked columns.

Hardware constraints honored (walrus/birverifier): psum matmul outputs stay
within one 2KB bank; gpsimd never touches PSUM; no f32r memsets. Tiles with
multiple DMA writers are split (k_own/v_my halves, kT 4-chunk groups)
because reader deps on DMA-written tiles are tile-granular. Output leaves
in a partition-major [128, 512] layout (fully contiguous store DMAs); the
host de-permutes.
"""

import os
import sys
import numpy as np

B, T, E, F = 4, 2048, 2048, 64
H = 1024          # q rows per core
NEG = -1e30
P0, P1 = 4, 12    # prefix chunks per slot (slot0 = blocks 0/1, slot1 = 3/2)

_cache = {}


def _ensure_path():
    if os.path.isdir("/opt/trn_rl_repo"):
        if "/opt/trn_rl_repo" not in sys.path:
            sys.path.insert(0, "/opt/trn_rl_repo")


def _build():
    _ensure_path()
    import concourse.bass as bass
    import concourse.bacc as bacc
    import concourse.mybir as mybir
    import concourse.tile as tile
    from concourse import masks

    dt = mybir.dt
    AF = mybir.ActivationFunctionType
    f32, f32r, bf16 = dt.float32, dt.float32r, dt.bfloat16

    nc = bacc.Bacc("TRN2", target_bir_lowering=False, debug=False, num_devices=8)

    # host-packed inputs (see _in_maps for layouts)
    xp = nc.dram_tensor("xp", [128, 16384], bf16, kind="ExternalInput")
    wp = nc.dram_tensor("wp", [128, 3072], bf16, kind="ExternalInput")
    bqkv = nc.dram_tensor("bqkv", [128, 2], f32, kind="ExternalInput")
    qrows = nc.dram_tensor("qrows", [2, 1024], f32r, kind="ExternalInput")
    krows = nc.dram_tensor("krows", [2, 1536], f32r, kind="ExternalInput")
    # output in partition-major layout [p, 256*slot + 64*m + f] where the
    # row is 512*slot + 128*m + p; the host de-permutes (free) — this keeps
    # the store DMA fully contiguous per partition
    out_d = nc.dram_tensor("out", [128, 512], f32, kind="ExternalOutput")

    RG = [[0, 1], [2, 3], [4, 5], [6, 7]]

    with tile.TileContext(nc) as tc:
        with (
            tc.tile_pool(name="const", bufs=1) as constp,
            tc.tile_pool(name="xpool", bufs=1) as xpool,
            tc.tile_pool(name="qkv", bufs=1) as qkvp,
            tc.tile_pool(name="dram", bufs=1, space="DRAM") as dram,
        ):
            # ---------------- input DMAs (front-load HWDGE in this order) ---
            # interleave w quarters with 2-e-tile x chunks so deliveries stay
            # just ahead of the matmul loop's consumption order
            wp_sb = constp.tile([128, 3072], bf16, tag="wp")
            x_tiles = []

            def xdma(j, c0=None, w=2048):
                if c0 is None:
                    c0 = 2048 * j
                xt = xpool.tile([128, w], bf16, tag=f"xt{j}")
                nc.sync.dma_start(out=xt[:], in_=xp[:, c0:c0 + w])
                x_tiles.append((c0, xt))

            def wdma(j):
                nc.sync.dma_start(
                    out=wp_sb[:, 768 * j:768 * (j + 1)],
                    in_=wp[:, 768 * j:768 * (j + 1)],
                )

            # first two e-tiles ride single-tile DMAs so the matmul loop
            # starts ~0.8us earlier
            wdma(0)
            xdma(8, c0=0, w=1024)
            xdma(9, c0=1024, w=1024)
            wdma(1)
            xdma(1)
            xdma(2)
            wdma(2)
            xdma(3)
            xdma(4)
            wdma(3)
            xdma(5)
            xdma(6)
            # last e-tiles ride single-tile DMAs too: e14's matmuls clear
            # before e15 lands, so the qk-stop trails the final x semaphore
            # by only one matmul
            xdma(10, c0=14336, w=1024)
            xdma(11, c0=15360, w=1024)

            x_tiles.sort(key=lambda t: t[0])

            def x_view(e):
                # [128, 1024] view of e-tile e
                base = 1024 * e
                for c0, xt in reversed(x_tiles):
                    if base >= c0:
                        return xt[:, base - c0:base - c0 + 1024]
                raise AssertionError

            bqkv_sb = constp.tile([128, 2], f32, tag="bqkv")
            nc.sync.dma_start(out=bqkv_sb[:], in_=bqkv[:, :])

            # q2 rows 64:66 (slot indicators) / kT_full rows 64:66 (cbias)
            q2 = qkvp.tile([66, H], f32r, tag="q2")
            nc.sync.dma_start(out=q2[64:66, :], in_=qrows[:, :])
            # gathered-K tiles split per 4-chunk group so each prefix pair
            # only depends on its own (small, early) load
            kT_g = []
            for j in range(3):
                kt = qkvp.tile([66, 512], f32r, tag=f"kt{j}", name=f"kt{j}")
                nc.sync.dma_start(out=kt[64:66, :],
                                  in_=krows[:, 512 * j:512 * (j + 1)])
                kT_g.append(kt)

            # ---------------- constants built on-device ----------------
            ident = constp.tile([128, 128], f32, tag="ident")
            masks.make_identity(nc, ident[:])
            # mbig[p, u] = 0 if (u - 512 - p) >= 0 else NEG ; diag-mask source
            mbig = constp.tile([128, 1024], f32, tag="mbig")
            nc.gpsimd.memset(mbig[:], 0.0)
            nc.gpsimd.affine_select(
                out=mbig[:], in_=mbig[:],
                compare_op=mybir.AluOpType.is_ge, fill=NEG,
                base=-512, channel_multiplier=-1, pattern=[[1, 1024]],
            )
            # local K / V-aug split in halves so nothing waits on a
            # whole-tile DMA dependency
            k_own = [qkvp.tile([66, 512], f32r, tag=f"k_own{j}",
                               name=f"k_own{j}") for j in range(2)]
            v_my = [qkvp.tile([128, 260], bf16, tag=f"v_my{j}",
                              name=f"v_my{j}") for j in range(2)]
            for j in range(2):
                nc.vector.memzero(k_own[j][64:66, :])
                for m in range(4):
                    nc.vector.memset(v_my[j][:, 65 * m + 64:65 * m + 65], 1.0)

            # activation-table warmup: load the Exp table before it matters
            warm = constp.tile([1, 2], bf16, tag="warm")
            nc.scalar.activation(warm[0:1, 0:1], bqkv_sb[0:1, 0:1], AF.Exp)

            # ---------------- projections ----------------
            # psum rows 0:64 = Q (lands partition-aligned in q2), rows
            # 64:128 = K (host packs Wq first; K moves base via a DMA that
            # runs in parallel with the K collective).
            ktmp = qkvp.tile([128, H], f32r, tag="ktmp")
            vT_sb = qkvp.tile([64, H], f32, tag="vT")

            with tc.tile_pool(name="pps", bufs=1, space="PSUM") as pps:
                ps_qk = [pps.tile([128, 512], f32, tag=f"psqk{i}",
                                  name=f"psqk{i}") for i in range(2)]
                ps_v = [pps.tile([64, 512], f32, tag=f"psv{i}",
                                 name=f"psv{i}") for i in range(2)]

                def mm_qk(e):
                    for i in range(2):
                        nc.tensor.matmul(
                            ps_qk[i][:],
                            lhsT=wp_sb[:, 192 * e:192 * e + 128],
                            rhs=x_view(e)[:, 512 * i:512 * (i + 1)],
                            start=(e == 0), stop=(e == 15),
                        )

                def mm_v(e):
                    for i in range(2):
                        nc.tensor.matmul(
                            ps_v[i][:],
                            lhsT=wp_sb[:, 192 * e + 128:192 * e + 192],
                            rhs=x_view(e)[:, 512 * i:512 * (i + 1)],
                            start=(e == 0), stop=(e == 15),
                        )

                # qk stream leads (its stop gates the K collective); v
                # matmuls backfill PE slack behind the x-DMA stream
                for e in range(16):
                    mm_qk(e)
                    if e >= 6:
                        mm_v(e - 6)
                for e in range(10, 16):
                    mm_v(e)
                # K first (feeds the collective), split across Act + DVE
                nc.scalar.activation(
                    ktmp[64:128, 0:512], ps_qk[0][64:128, :],
                    AF.Identity, bias=bqkv_sb[64:128, 0:1],
                )
                nc.vector.tensor_scalar_add(
                    ktmp[64:128, 512:1024], ps_qk[1][64:128, :],
                    bqkv_sb[64:128, 0:1],
                )
                nc.scalar.activation(
                    q2[0:64, 0:512], ps_qk[0][0:64, :],
                    AF.Identity, bias=bqkv_sb[0:64, 0:1],
                )
                nc.scalar.activation(
                    q2[0:64, 512:1024], ps_qk[1][0:64, :],
                    AF.Identity, bias=bqkv_sb[0:64, 0:1],
                )
                nc.vector.tensor_scalar_add(
                    vT_sb[:, 0:512], ps_v[0][:], bqkv_sb[0:64, 1:2],
                )
                nc.vector.tensor_scalar_add(
                    vT_sb[:, 512:1024], ps_v[1][:], bqkv_sb[0:64, 1:2],
                )

            # ---------------- K collective (start ASAP) ----------------
            # halved by columns: each half starts as soon as its bias-add
            # (Act for h0, DVE for h1) lands, pipelining store/gather/load
            bk_h = [dram.tile([64, 512], f32r, tag=f"bk{j}", name=f"bk{j}")
                    for j in range(2)]
            gk_h = [dram.tile([128, 512], f32r, tag=f"gk{j}", name=f"gk{j}")
                    for j in range(2)]
            bv0_d = dram.tile([128, 260], bf16, tag="bv0d")
            bv1_d = dram.tile([128, 260], bf16, tag="bv1d")
            gv0_d = dram.tile([256, 260], bf16, tag="gv0d")
            gv1_d = dram.tile([256, 260], bf16, tag="gv1d")

            for j in range(2):
                nc.sync.dma_start(out=bk_h[j][:],
                                  in_=ktmp[64:128, 512 * j:512 * (j + 1)])
                # K into contract-base-0 tile for the diag matmuls
                # (partition move needs a DMA)
                nc.sync.dma_start(out=k_own[j][0:64, :],
                                  in_=ktmp[64:128, 512 * j:512 * (j + 1)])
            for j in range(2):
                if os.environ.get("NOCC"):
                    # timing-model stub: emulate the pair-gather's movement
                    nc.sync.dma_start(
                        out=gk_h[j][:].rearrange("(t p) f -> t p f", t=2),
                        in_=bk_h[j][:].unsqueeze(0).broadcast_to((2, 64, 512)),
                    )
                else:
                    nc.gpsimd.collective_compute(
                        "AllGather", mybir.AluOpType.bypass, replica_groups=RG,
                        ins=[bk_h[j][:].opt()], outs=[gk_h[j][:].opt()],
                    )

            # gathered K: global chunks 0..3 = A's h0, 4..7 = B's h0,
            # 8..11 = B's h1 (A local cols: 0:512=blk0, 512:1024=blk3;
            # B: blk1, blk2). Queued on SP BEFORE the V store so the
            # exp-critical kT loads don't wait behind the V chain.
            nc.sync.dma_start(out=kT_g[0][0:64, :], in_=gk_h[0][0:64, :])
            nc.sync.dma_start(out=kT_g[1][0:64, :], in_=gk_h[0][64:128, :])
            nc.sync.dma_start(out=kT_g[2][0:64, :], in_=gk_h[1][64:128, :])

            # ---------------- attention ----------------
            with (
                tc.tile_pool(name="ot", bufs=1, space="PSUM") as otp,
                tc.tile_pool(name="sb", bufs=8) as sbp,
                tc.tile_pool(name="pd", bufs=8) as pdp,
                tc.tile_pool(name="ob", bufs=3) as obp,
            ):
                out_ps = [otp.tile([65, 512], f32, tag=f"ot{i}", name=f"ot{i}")
                          for i in range(2)]

                v_ga = qkvp.tile([128, 780], bf16, tag="v_ga")

                def vgather(half):
                    # V collective in halves (chunks 0..3 / 4..7 local) so
                    # the early chunks reach v_ga sooner; all on Pool SWDGE
                    bv = (bv0_d, bv1_d)[half]
                    gv = (gv0_d, gv1_d)[half]
                    nc.gpsimd.dma_start(out=bv[:], in_=v_my[half][:])
                    if os.environ.get("NOCC"):
                        # stub: emulate the pair-gather's data movement
                        nc.gpsimd.dma_start(
                            out=gv[:].rearrange("(t p) f -> t p f", t=2),
                            in_=bv[:].unsqueeze(0).broadcast_to((2, 128, 260)),
                        )
                    else:
                        nc.gpsimd.collective_compute(
                            "AllGather", mybir.AluOpType.bypass,
                            replica_groups=RG,
                            ins=[bv[:].opt()], outs=[gv[:].opt()],
                        )
                    # gathered V-aug -> v_ga: half0 gives global chunks 0..7
                    # (both cores' first halves), half1 gives 8..11 (B's
                    # second half; A's second half is unused)
                    if half == 0:
                        nc.gpsimd.dma_start(out=v_ga[:, 0:260], in_=gv[0:128, :])
                        nc.gpsimd.dma_start(out=v_ga[:, 260:520], in_=gv[128:256, :])
                    else:
                        nc.gpsimd.dma_start(out=v_ga[:, 520:780], in_=gv[128:256, :])

                diag_p = []
                vtp_ctx = tc.tile_pool(name="vtp", bufs=1, space="PSUM")
                vtp = vtp_ctx.__enter__()
                dlg_ctx = tc.tile_pool(name="dlg", bufs=4, space="PSUM")
                dlgp = dlg_ctx.__enter__()

                def diag_chunk(i, k):
                    # diagonal logits + masked exp (local data only)
                    lg = dlgp.tile([128, 512], f32, tag="dlg")
                    nc.tensor.matmul(
                        lg[:],
                        lhsT=k_own[i][:, 128 * k:128 * (k + 1)],
                        rhs=q2[:, 512 * i:512 * (i + 1)],
                        start=True, stop=True,
                    )
                    # in-place causal mask on the cols where t < s
                    w = 128 * (k + 1)
                    nc.vector.tensor_add(
                        lg[:, 0:w], lg[:, 0:w],
                        mbig[:, 512 - 128 * k:512 - 128 * k + w],
                    )
                    p_sb = pdp.tile([128, 512], bf16, tag="p")
                    nc.scalar.activation(p_sb[:], lg[:], AF.Exp, scale=0.125)
                    diag_p.append((i, k, p_sb))

                def diag_avs(i):
                    for ii, k, p_sb in diag_p:
                        if ii != i:
                            continue
                        nc.tensor.matmul(
                            out_ps[i][:],
                            lhsT=v_my[i][:, 65 * k:65 * k + 65],
                            rhs=p_sb[:],
                            start=(k == 0), stop=False,
                        )

                # diag logits lead (they gate the exp stream); V transposes
                # write disjoint columns of ONE psum tile (no WAR
                # serialization), with copies on Pool so DVE stays free for
                # the diag masks
                pt = vtp.tile([128, 512], f32, tag="vt")
                for i, k in ((0, 0), (0, 1), (0, 2), (0, 3)):
                    diag_chunk(i, k)
                for m in range(8):
                    nc.tensor.transpose(
                        pt[:, 64 * m:64 * (m + 1)],
                        vT_sb[:, 128 * m:128 * (m + 1)], ident[0:64, 0:64]
                    )
                for j in range(2):
                    # one strided copy per half (no write-after-read ladder),
                    # on the Act engine's idle window (Pool can't read PSUM)
                    nc.scalar.copy(
                        v_my[j][:].rearrange("p (c f) -> p c f", c=4)[:, :, 0:64],
                        pt[:, 256 * j:256 * (j + 1)]
                        .rearrange("p (c f) -> p c f", c=4),
                    )
                for i, k in ((1, 0), (1, 1), (1, 2), (1, 3)):
                    diag_chunk(i, k)
                vgather(0)
                vgather(1)
                diag_avs(0)

                # ---- prefix chunks: logits+exp stream decoupled from the
                # AV matmuls so the exp stream never stalls on v_ga ----
                dlg_ctx.__exit__(None, None, None)
                vtp_ctx.__exit__(None, None, None)
                lgp_ctx = tc.tile_pool(name="lg", bufs=2, space="PSUM")
                lgp = lgp_ctx.__enter__()

                def pl(i, gg, n):
                    # logits for n chunks (gg..gg+n-1) of slot i, one exp
                    lg = lgp.tile([128, 1024], f32, tag="lg")
                    for d in range(n):
                        g = gg + d
                        kt = kT_g[g // 4]
                        c0 = 128 * (g % 4)
                        nc.tensor.matmul(
                            lg[:, 512 * d:512 * (d + 1)],
                            lhsT=kt[:, c0:c0 + 128],
                            rhs=q2[:, 512 * i:512 * (i + 1)],
                            start=True, stop=True,
                        )
                    p_sb = sbp.tile([128, 1024], bf16, tag="p")
                    nc.scalar.activation(p_sb[:, 0:512 * n], lg[:, 0:512 * n],
                                         AF.Exp, scale=0.125)
                    return p_sb

                def pa(i, gg, n, p_sb, last):
                    for d in range(n):
                        g = gg + d
                        nc.tensor.matmul(
                            out_ps[i][:],
                            lhsT=v_ga[:, 65 * g:65 * g + 65],
                            rhs=p_sb[:, 512 * d:512 * (d + 1)],
                            start=False, stop=(g == last),
                        )

                def finalize(i, fbufs=2, nout=1):
                    # copy, transpose, normalize, store. For the last slot:
                    # the denominator row is reciprocated ONCE before the
                    # transposes (so the transposed tile carries 1/denom as
                    # the multiply scalar) and the numerator copy rides the
                    # idle Act engine.
                    oc = obp.tile([65, 512], f32, tag="oc")
                    if nout == 2:
                        nc.scalar.copy(oc[:, 0:256], out_ps[i][:, 0:256])
                        nc.vector.tensor_copy(oc[:, 256:512],
                                              out_ps[i][:, 256:512])
                    else:
                        nc.vector.tensor_copy(oc[:], out_ps[i][:])
                    o_cat = obp.tile([128, 256], f32, tag="ocat")
                    with tc.tile_pool(name=f"ft{i}", bufs=fbufs,
                                      space="PSUM") as ftp:
                        for m in range(4):
                            ft = ftp.tile([128, 65], f32, tag="ft")
                            nc.tensor.transpose(
                                ft[:], oc[:, 128 * m:128 * (m + 1)],
                                ident[0:65, 0:65],
                            )
                            rc = obp.tile([128, 1], f32, tag="rc")
                            nc.vector.reciprocal(rc[:], ft[:, 64:65])
                            nc.vector.tensor_scalar_mul(
                                o_cat[:, 64 * m:64 * (m + 1)], ft[:, 0:64],
                                rc[:, 0:1],
                            )
                            if nout == 2 and m % 2 == 1:
                                nc.sync.dma_start(
                                    out=out_d[:, 256 * i + 128 * (m // 2):
                                              256 * i + 128 * (m // 2) + 128],
                                    in_=o_cat[:, 128 * (m // 2):
                                              128 * (m // 2) + 128],
                                )
                    if nout == 1:
                        nc.sync.dma_start(
                            out=out_d[:, 256 * i:256 * (i + 1)], in_=o_cat[:],
                        )

                # slot0 (short) first; slot1 logits keep the exp stream fed
                # while slot0's AVs + finalize ride in the PE slack
                p0t = pl(0, 0, 2)
                p0s = pl(0, 2, 2)
                diag_avs(1)
                p1a = pl(1, 0, 2)
                p1b = pl(1, 2, 2)
                pa(0, 0, 2, p0t, -1)
                pa(0, 2, 2, p0s, P0 - 1)
                p1c = pl(1, 4, 2)
                finalize(0)
                p1d = pl(1, 6, 2)
                pa(1, 0, 2, p1a, -1)
                p1e = pl(1, 8, 2)
                pa(1, 2, 2, p1b, -1)
                p1f = pl(1, 10, 2)
                pa(1, 4, 2, p1c, -1)
                pa(1, 6, 2, p1d, -1)
                pa(1, 8, 2, p1e, -1)
                pa(1, 10, 2, p1f, P1 - 1)
                lgp_ctx.__exit__(None, None, None)
                finalize(1, fbufs=4, nout=2)

    nc.compile()
    return nc


def _in_maps(x, Wq, bq, Wk, bk, Wv, bv):
    import ml_dtypes

    bf16 = ml_dtypes.bfloat16
    # wcat: per e-tile 192 cols = [Wq(64) | Wk(64) | Wv(64)] transposed
    wcat = np.concatenate([Wq, Wk, Wv], axis=0).T.astype(np.float32)  # [E,192]
    wp = np.ascontiguousarray(
        wcat.reshape(16, 128, 192).transpose(1, 0, 2).reshape(128, 3072)
    ).astype(bf16)
    bqkv = np.zeros((128, 2), np.float32)
    bqkv[0:64, 0] = bq
    bqkv[64:128, 0] = bk
    bqkv[0:64, 1] = bv

    maps = []
    for core in range(8):
        b, h = core // 2, core % 2
        # A (h=0): blocks [0, 3]; B (h=1): blocks [1, 2]
        blocks = (0, 3) if h == 0 else (1, 2)
        xr = np.concatenate(
            [x[b, 512 * blk:512 * (blk + 1), :] for blk in blocks], axis=0
        )  # [1024, E]
        xT = xr.T.astype(np.float32)  # [E, 1024]
        xpk = np.ascontiguousarray(
            xT.reshape(4, 4, 128, 1024).transpose(2, 0, 1, 3).reshape(128, 16384)
        ).astype(bf16)
        # slot indicator rows for q (contract rows 64, 65)
        qr = np.zeros((2, 1024), np.float32)
        qr[0, 0:512] = 1.0
        qr[1, 512:1024] = 1.0
        # cbias rows for kT_full: row j, chunk g killed if g >= prefix limit
        kr = np.full((2, 1536), NEG, np.float32)
        lim0 = 0 if h == 0 else 4    # slot0 = block 0 (A) / block 1 (B)
        lim1 = 12 if h == 0 else 8   # slot1 = block 3 (A) / block 2 (B)
        kr[0, 0:128 * lim0] = 0.0
        kr[1, 0:128 * lim1] = 0.0
        maps.append({
            "xp": xpk, "wp": wp, "bqkv": bqkv,
            "qrows": qr, "krows": kr,
        })
    return maps


def kernel(x, Wq, bq, Wk, bk, Wv, bv, _want_time=False):
    _ensure_path()
    from concourse.bass_utils import run_bass_kernel_spmd

    if "nc" not in _cache:
        _cache["nc"] = _build()
    nc = _cache["nc"]
    maps = _in_maps(x, Wq, bq, Wk, bk, Wv, bv)
    res = run_bass_kernel_spmd(nc, maps, core_ids=list(range(8)),
                               trace=bool(int(os.environ.get("KTRACE", "0"))))
    _cache["last"] = res
    out = np.empty((B, T, F), np.float32)
    for core in range(8):
        b, h = core // 2, core % 2
        blocks = (0, 3) if h == 0 else (1, 2)
        # device layout: out_d[p, 256*slot + 64*m + f] = row 128*m + p of slot
        oc = res.results[core]["out"].reshape(128, 2, 4, F)
        for slot, blk in enumerate(blocks):
            out[b, 512 * blk:512 * (blk + 1), :] = (
                oc[:, slot, :, :].transpose(1, 0, 2).reshape(512, F)
            )
    return out
